# revision 2
# baseline (speedup 1.0000x reference)
"""Trainium2 Bass kernel v2 for the attention-scoring module.

    q = query @ Wq.T + bq                               # (B, D)
    ref[b,d,k] = sum_e enc[k,b,e] * Wref[d,e] + bref[d]
    u[b,k] = sum_d v[d] * tanh(ref[b,d,k] + q[b,d])
    out = 10 * tanh(u)                                  # (B, K)

Data-parallel over batch: core c owns b in [32c, 32c+32).

v2 changes vs baseline:
  - sign(v) folded into Wref/Wq/bref/bq rows (tanh odd), then d's sorted
    by |v| and paired (rank 2i, 2i+1) -> chunks c0/c1 with ratio=v1/v0
    in [1, ~1.9].  The d-combine is then w' = t0 + ratio*t1 (one
    tensor_scalar + one tensor_tensor, both 2x) and the strip matmul's
    stationary column is v0 (not ones) - halves DVE combine cost.
  - 16 of 32 b-rows use fp8(e4m3) DoubleRow matmuls: contraction 256 in
    one instruction at 2x rate; enc scaled x16, Wref x512, descale via
    the activation's scale input.  Error budget ~1.6e-2 < 2e-2.
  - t/w tiles fp16 (same speed, more precision margin than bf16).
"""

import os
import sys

import numpy as np

os.environ.setdefault("JAX_COMPILATION_CACHE_DIR", "/tmp/jaxcache")

for _p in ("/opt/trn_rl_repo", "/opt/pypackages"):
    if _p not in sys.path:
        sys.path.append(_p)

import ml_dtypes

E = 256
D = 256
K = 2048
B = 256
NCORES = 8
BL = B // NCORES          # 32 batch rows per core
NB8 = 12                  # b-rows (per core) on the fp8 path
NBB = BL - NB8            # b-rows on the bf16 path
N8 = NB8 * K
NB = NBB * K
SLAB_B = 4                # b-rows per enc DMA slab
SLAB_N = SLAB_B * K       # 8192
C_CLIP = 10.0
S_ENC = 16.0
S_W = 512.0
DESCALE = 1.0 / (S_ENC * S_W)

_compiled = None
last_exec_time_ns = None
last_results = None


def _build():
    from concourse import bacc, bass, tile
    from concourse.alu_op_type import AluOpType as ALU

    mybir = bass.mybir
    dt = mybir.dt
    AF = mybir.ActivationFunctionType

    nc = bacc.Bacc("TRN2", target_bir_lowering=False, debug=False,
                   num_devices=NCORES)

    # fp8 enc for b 0..15: rows 0:128 = stream A (e 0:128), 128:256 = B
    enc8_t = nc.declare_dram_parameter("enc8", [E, N8], dt.float8e4, isOutput=False)
    # bf16 enc for b 16..31, two row-halves like the baseline
    encb_t = nc.declare_dram_parameter("encb", [E, NB], dt.bfloat16, isOutput=False)
    cf32_t = nc.declare_dram_parameter("cf32", [128, 581], dt.float32, isOutput=False)
    cbf16_t = nc.declare_dram_parameter("cbf16", [128, 512], dt.bfloat16, isOutput=False)
    cfp8_t = nc.declare_dram_parameter("cfp8", [128, 512], dt.float8e4, isOutput=False)
    cfp16_t = nc.declare_dram_parameter("cfp16", [128, 1], dt.float16, isOutput=False)
    out_p = nc.declare_dram_parameter("out", [128, 512], dt.float32, isOutput=True)

    with tile.TileContext(nc) as tc:
        with (
            tc.tile_pool(name="const", bufs=1) as constp,
            tc.tile_pool(name="enc", bufs=3) as encp,
            tc.tile_pool(name="tt", bufs=10) as tp,
            tc.tile_pool(name="tail", bufs=2) as tailp,
            tc.tile_pool(name="psum_m", bufs=3, space="PSUM") as pmp,
            tc.tile_pool(name="psum_s", bufs=2, space="PSUM") as psp,
        ):
            # ---- constants: packed DMAs ----
            cf32_sb = constp.tile([128, 581], dt.float32)
            cbf16_sb = constp.tile([128, 512], dt.bfloat16)
            cfp8_sb = constp.tile([128, 512], dt.float8e4)
            cfp16_sb = constp.tile([128, 1], dt.float16)
            bias_sb = constp.tile([128, 2 * BL], dt.float32)   # [:, dc*32 + b]
            u_sb = constp.tile([128, 512], dt.float32)         # [b*4+jj, kk]
            nc.sync.dma_start(cf32_sb[:], cf32_t[:])
            nc.sync.dma_start(cbf16_sb[:], cbf16_t[:])
            nc.sync.dma_start(cfp8_sb[:], cfp8_t[:])
            nc.sync.dma_start(cfp16_sb[:], cfp16_t[:])

            wq_sb = cf32_sb[:, 0:512]        # [:, (ec*2+dc)*128 + d]
            query_sb = cf32_sb[:, 512:576]   # [:, ec*32 + b]
            cbias_sb = cf32_sb[:, 576:578]
            ratio_sb = cf32_sb[:, 578:579]   # v1/v0 per pair-partition
            wref_sb = cbf16_sb[:, 0:512]     # bf16 W [(ec*2+dc)*128 + d]
            w8_sb = cfp8_sb[:, 0:512]        # fp8 W pairs [dc*256 + i*128 + m]
            v0_sb = cfp16_sb[:, 0:1]         # fp16 |v| even-rank column

            # ---- enc slab loading ----
            # fp8 b-rows first (slabs of 4 b), then bf16 b-rows
            def alloc_slab8(s):
                return encp.tile([128, 2 * SLAB_N], dt.float8e4, tag="enc8",
                                 name=f"enc8_s{s}")

            def emit_pieces8(t8, s, q0, q1, pieces=SLAB_B, dep=None):
                w = SLAB_N // pieces
                for q in range(q0, q1):
                    for ec in range(2):
                        ins = nc.sync.dma_start(
                            t8[:, ec * SLAB_N + q * w:ec * SLAB_N + (q + 1) * w],
                            enc8_t[ec * 128:(ec + 1) * 128,
                                   s * SLAB_N + q * w:s * SLAB_N + (q + 1) * w])
                        if dep is not None:
                            tile.add_dep_helper(ins.ins, dep.ins,
                                                reason="defer enc prefetch")

            def alloc_slabb(s):
                return [encp.tile([128, SLAB_N], dt.bfloat16, tag=f"encb{ec}",
                                  name=f"encb{ec}_s{s}")
                        for ec in range(2)]

            def emit_piecesb(tiles, s, q0, q1, pieces=SLAB_B, dep=None):
                w = SLAB_N // pieces
                for q in range(q0, q1):
                    for ec in range(2):
                        ins = nc.sync.dma_start(
                            tiles[ec][:, q * w:(q + 1) * w],
                            encb_t[ec * 128:(ec + 1) * 128,
                                   s * SLAB_N + q * w:s * SLAB_N + (q + 1) * w])
                        if dep is not None:
                            tile.add_dep_helper(ins.ins, dep.ins,
                                                reason="defer enc prefetch")

            slab0 = alloc_slab8(0)
            emit_pieces8(slab0, 0, 0, 1)

            # ---- q_rawT = (query @ Wq'.T).T per dc-chunk, + (bref' + bq') ----
            for dc in range(2):
                qps = psp.tile([128, BL], dt.float32, tag="st")
                for ec in range(2):
                    nc.tensor.matmul(
                        qps[:],
                        wq_sb[:, (ec * 2 + dc) * 128:(ec * 2 + dc + 1) * 128],
                        query_sb[:, ec * BL:(ec + 1) * BL],
                        start=(ec == 0), stop=(ec == 1),
                    )
                nc.vector.tensor_scalar_add(bias_sb[:, dc * BL:(dc + 1) * BL],
                                            qps[:], cbias_sb[:, dc:dc + 1])

            # ---- per-(b, kp) tail: combine + strips ----
            def emit_w(tts, b):
                # w' = t0 + ratio * t1  (TS 2x + TT 2x), fp16
                w1 = tp.tile([128, 1024], dt.float16, tag="w1", bufs=3)
                nc.vector.tensor_scalar_mul(w1[:], tts[1][:], ratio_sb[:, 0:1])
                w = tp.tile([128, 1024], dt.float16, tag="w", bufs=3)
                nc.vector.tensor_add(w[:], w1[:], tts[0][:])
                return w

            def emit_strips(st4, w, b, kp):
                for kb in range(2):
                    jj = kp * 2 + kb
                    nc.tensor.matmul(
                        st4[32 * jj:32 * jj + 1, :],
                        v0_sb,
                        w[:, kb * 512:(kb + 1) * 512],
                        start=True, stop=True,
                        skip_group_check=True,
                        tile_position=(0, 32 * jj),
                    )
                if kp == 1:
                    sp = tailp.tile([128, 512], dt.float32, tag="sp")
                    nc.vector.tensor_copy(sp[:], st4[:])
                    nc.gpsimd.dma_start(u_sb[4 * b:4 * b + 4, :],
                                        sp[0:128:32, :])

            t6 = constp.tile([128, 512], dt.float32)
            o6 = constp.tile([128, 512], dt.float32)

            def emit_final(half):
                rows = slice(64 * half, 64 * half + 64)
                nc.scalar.activation(t6[rows, :], u_sb[rows, :], AF.Tanh)
                nc.vector.tensor_scalar_mul(o6[rows, :], t6[rows, :], C_CLIP)
                nc.sync.dma_start(out_p[rows, :], o6[rows, :])

            pend = []
            prev_mm = None

            def tail_pump():
                while len(pend) > 2:
                    emit_strips(*pend.pop(0))

            # ---- unified main loop: fp8 / bf16 slabs interleaved ----
            NSLAB8 = N8 // SLAB_N
            NSLABB = NB // SLAB_N
            SLAB_SEQ = [("8", 0), ("b", 0), ("8", 1), ("b", 1), ("8", 2),
                        ("b", 2), ("b", 3), ("b", 4)]
            assert NSLAB8 == 3 and NSLABB == 5

            def alloc_and_prefetch(si, dep):
                typ, s = SLAB_SEQ[si]
                if typ == "8":
                    t = alloc_slab8(s)
                    emit_pieces8(t, s, 0, 2, 2, dep=dep)
                else:
                    t = alloc_slabb(s)
                    emit_piecesb(t, s, 0, 2, 2, dep=dep)
                return t

            cur_slab = slab0
            for si, (typ, s) in enumerate(SLAB_SEQ):
                nxt_slab = None
                for b_in in range(SLAB_B):
                    b = (SLAB_B * s + b_in) if typ == "8" else (NB8 + SLAB_B * s + b_in)
                    st4 = psp.tile([128, 512], dt.float32, tag="st")
                    if typ == "8":
                        enc_pair = cur_slab[:].rearrange("p (two n) -> p two n", two=2)
                    for kp in range(2):
                        if si == 0 and (b_in, kp) == (0, 1):
                            emit_pieces8(cur_slab, 0, 1, 2, dep=prev_mm)
                        if si == 0 and (b_in, kp) == (1, 0):
                            emit_pieces8(cur_slab, 0, 2, 4, dep=prev_mm)
                        pf_at = 2 if si == 0 else 1
                        if (b_in, kp) == (pf_at, 0) and si + 1 < len(SLAB_SEQ):
                            nxt_slab = alloc_and_prefetch(si + 1, prev_mm)
                        first_mm = None
                        tts = []
                        for dc in range(2):
                            psd = pmp.tile([128, 1024], dt.float32, tag="psd")
                            if typ == "8":
                                for kb in range(4):
                                    nseg = b_in * K + kp * 1024 + kb * 256
                                    ins = nc.tensor.matmul(
                                        psd[:, kb * 256:(kb + 1) * 256],
                                        w8_sb[:, dc * 256:(dc + 1) * 256].rearrange(
                                            "p (two m) -> p two m", two=2),
                                        enc_pair[:, :, nseg:nseg + 256],
                                        start=True, stop=True,
                                        perf_mode=mybir.MatmulPerfMode.DoubleRow,
                                        skip_group_check=True,
                                    )
                                    if first_mm is None:
                                        first_mm = ins
                            else:
                                for ec in range(2):
                                    for kb in range(2):
                                        nseg = b_in * K + kp * 1024 + kb * 512
                                        ins = nc.tensor.matmul(
                                            psd[:, kb * 512:(kb + 1) * 512],
                                            wref_sb[:, (ec * 2 + dc) * 128:(ec * 2 + dc + 1) * 128],
                                            cur_slab[ec][:, nseg:nseg + 512],
                                            start=(ec == 0), stop=(ec == 1),
                                            skip_group_check=True,
                                        )
                                        if first_mm is None:
                                            first_mm = ins
                            ttile = tp.tile([128, 1024], dt.float16, tag="tt")
                            nc.scalar.activation(
                                ttile[:], psd[:], AF.Tanh,
                                bias=bias_sb[:, dc * BL + b:dc * BL + b + 1],
                                scale=(DESCALE if typ == "8" else 1.0))
                            tts.append(ttile)
                        w = emit_w(tts, b)
                        pend.append((st4, w, b, kp))
                        tail_pump()
                        prev_mm = first_mm
                        if (si, b_in, kp) == (5, 1, 0):
                            emit_final(0)
                cur_slab = nxt_slab
            for args in pend:
                emit_strips(*args)
            emit_final(1)

    nc.compile()
    return nc


def _prep_inputs(encoder_output, query, Wq, bq, Wref, bref, v):
    bf16 = ml_dtypes.bfloat16
    fp16 = np.float16
    e4 = ml_dtypes.float8_e4m3fn if hasattr(ml_dtypes, "float8_e4m3fn") else ml_dtypes.float8_e4m3

    v = np.asarray(v, np.float32)
    sgn = np.where(v >= 0, 1.0, -1.0).astype(np.float32)
    va = np.abs(v)
    order = np.argsort(va, kind="stable")
    c0_idx, c1_idx = order[0::2], order[1::2]
    v0, v1 = va[c0_idx], va[c1_idx]
    ratio = (v1 / v0).astype(np.float32)
    perm = np.concatenate([c0_idx, c1_idx])          # new d order (dc-major)

    Wp = (np.asarray(Wref, np.float32) * sgn[:, None])[perm]     # (256, 256)
    Wqp = (np.asarray(Wq, np.float32) * sgn[:, None])[perm]
    cbias = (np.asarray(bref, np.float32) + np.asarray(bq, np.float32)) * sgn
    cbias = cbias[perm]

    def chunk4(w):                                   # (E, 256d) -> (512, 128)
        return np.ascontiguousarray(
            w.reshape(2, 128, 2, 128).transpose(0, 2, 1, 3).reshape(512, 128))

    def pack(w4):                                    # (4*128, X) -> (128, 4*X)
        x = w4.shape[1]
        return w4.reshape(4, 128, x).transpose(1, 0, 2).reshape(128, 4 * x)

    # bf16 W pack: WT (E, D') where D' columns are [c0 | c1]
    WT = np.ascontiguousarray(Wp.T)                  # (E, 256) cols dc-major
    wref_p = pack(chunk4(WT)).astype(bf16)           # (128, 512)
    wq_p = pack(chunk4(np.ascontiguousarray(Wqp.T))) # (128, 512) f32

    # fp8 W pairs: per dc, lhsT[e, i, m] = Wp[dc*128+m, i*128+e] * S_W
    w8 = np.empty((128, 512), np.float32)
    for dc in range(2):
        chunk = Wp[dc * 128:(dc + 1) * 128] * S_W    # (128 d, 256 e)
        for i in range(2):
            # (e, m) block
            w8[:, dc * 256 + i * 128:dc * 256 + (i + 1) * 128] = \
                chunk[:, i * 128:(i + 1) * 128].T
    w8 = np.clip(w8, -240.0, 240.0).astype(e4)

    cbias_p = cbias.reshape(2, 128).T                # (128, 2)
    ratio_p = ratio.reshape(128, 1)
    v0_p = v0.reshape(128, 1).astype(fp16)
    queryT = np.ascontiguousarray(np.asarray(query, np.float32).T)  # (E, B)

    enc = np.asarray(encoder_output, np.float32)     # (K, B, E)
    encT = enc.transpose(2, 1, 0)                    # (E, B, K) view

    in_maps = []
    for c in range(NCORES):
        bs = slice(c * BL, (c + 1) * BL)
        enc_c = encT[:, bs, :]                       # (E, 32, K)
        enc8 = np.ascontiguousarray(enc_c[:, 0:NB8, :]).reshape(E, N8)
        enc8 = np.clip(enc8 * S_ENC, -240.0, 240.0).astype(e4)
        encb = np.ascontiguousarray(enc_c[:, NB8:, :]).reshape(E, NB).astype(bf16)

        q_c = queryT[:, bs]                          # (256, 32)
        q_p = q_c.reshape(2, 128, BL).transpose(1, 0, 2).reshape(128, 2 * BL)
        cf32 = np.ascontiguousarray(np.concatenate(
            [wq_p, q_p, cbias_p, ratio_p,
             np.zeros((128, 2), np.float32)], axis=1), dtype=np.float32)
        in_maps.append({
            "enc8": enc8,
            "encb": encb,
            "cf32": cf32,
            "cbf16": wref_p,
            "cfp8": w8,
            "cfp16": v0_p,
        })
    return in_maps


def kernel(**inputs):
    global _compiled, last_exec_time_ns, last_results
    from concourse import bass_utils

    if _compiled is None:
        _compiled = _build()
    nc = _compiled

    in_maps = _prep_inputs(**inputs)
    res = bass_utils.run_bass_kernel_spmd(nc, in_maps, core_ids=list(range(NCORES)))
    last_exec_time_ns = res.exec_time_ns
    last_results = res
    out = np.concatenate(
        [r["out"].reshape(BL, K) for r in res.results], axis=0)
    return out


# revision 3
# speedup vs baseline: 1.0027x; 1.0027x over previous
"""Trainium2 Bass kernel v2 for the attention-scoring module.

    q = query @ Wq.T + bq                               # (B, D)
    ref[b,d,k] = sum_e enc[k,b,e] * Wref[d,e] + bref[d]
    u[b,k] = sum_d v[d] * tanh(ref[b,d,k] + q[b,d])
    out = 10 * tanh(u)                                  # (B, K)

Data-parallel over batch: core c owns b in [32c, 32c+32).

v2 changes vs baseline:
  - sign(v) folded into Wref/Wq/bref/bq rows (tanh odd), then d's sorted
    by |v| and paired (rank 2i, 2i+1) -> chunks c0/c1 with ratio=v1/v0
    in [1, ~1.9].  The d-combine is then w' = t0 + ratio*t1 (one
    tensor_scalar + one tensor_tensor, both 2x) and the strip matmul's
    stationary column is v0 (not ones) - halves DVE combine cost.
  - 12 of 32 b-rows use fp8(e4m3) DoubleRow matmuls: contraction 256 in
    one instruction at 2x rate; enc scaled x16, Wref x512, descale via
    the activation's scale input.  Measured error 1.67e-2 < 2e-2 gate.
  - t/w tiles fp16 (same speed, more precision margin than bf16).
"""

import os
import sys

import numpy as np

os.environ.setdefault("JAX_COMPILATION_CACHE_DIR", "/tmp/jaxcache")

for _p in ("/opt/trn_rl_repo", "/opt/pypackages"):
    if _p not in sys.path:
        sys.path.append(_p)

import ml_dtypes

E = 256
D = 256
K = 2048
B = 256
NCORES = 8
BL = B // NCORES          # 32 batch rows per core
NB8 = 12                  # b-rows (per core) on the fp8 path
NBB = BL - NB8            # b-rows on the bf16 path
N8 = NB8 * K
NB = NBB * K
SLAB_B = 4                # b-rows per enc DMA slab
SLAB_N = SLAB_B * K       # 8192
C_CLIP = 10.0
S_ENC = 16.0
S_W = 512.0
DESCALE = 1.0 / (S_ENC * S_W)

_compiled = None
last_exec_time_ns = None
last_results = None


def _build():
    from concourse import bacc, bass, tile
    from concourse.alu_op_type import AluOpType as ALU

    mybir = bass.mybir
    dt = mybir.dt
    AF = mybir.ActivationFunctionType

    nc = bacc.Bacc("TRN2", target_bir_lowering=False, debug=False,
                   num_devices=NCORES)

    # fp8 enc for b 0..15: rows 0:128 = stream A (e 0:128), 128:256 = B
    enc8_t = nc.declare_dram_parameter("enc8", [E, N8], dt.float8e4, isOutput=False)
    # bf16 enc for b 16..31, two row-halves like the baseline
    encb_t = nc.declare_dram_parameter("encb", [E, NB], dt.bfloat16, isOutput=False)
    cf32_t = nc.declare_dram_parameter("cf32", [128, 581], dt.float32, isOutput=False)
    cbf16_t = nc.declare_dram_parameter("cbf16", [128, 512], dt.bfloat16, isOutput=False)
    cfp8_t = nc.declare_dram_parameter("cfp8", [128, 512], dt.float8e4, isOutput=False)
    cfp16_t = nc.declare_dram_parameter("cfp16", [128, 1], dt.float16, isOutput=False)
    out_p = nc.declare_dram_parameter("out", [128, 512], dt.float32, isOutput=True)

    with tile.TileContext(nc) as tc:
        with (
            tc.tile_pool(name="const", bufs=1) as constp,
            tc.tile_pool(name="enc", bufs=3) as encp,
            tc.tile_pool(name="tt", bufs=10) as tp,
            tc.tile_pool(name="tail", bufs=2) as tailp,
            tc.tile_pool(name="psum_m", bufs=3, space="PSUM") as pmp,
            tc.tile_pool(name="psum_s", bufs=2, space="PSUM") as psp,
        ):
            # ---- constants: packed DMAs ----
            cf32_sb = constp.tile([128, 581], dt.float32)
            cbf16_sb = constp.tile([128, 512], dt.bfloat16)
            cfp8_sb = constp.tile([128, 512], dt.float8e4)
            cfp16_sb = constp.tile([128, 1], dt.float16)
            bias_sb = constp.tile([128, 2 * BL], dt.float32)   # [:, dc*32 + b]
            u_sb = constp.tile([128, 512], dt.float32)         # [b*4+jj, kk]
            nc.sync.dma_start(cf32_sb[:], cf32_t[:])
            nc.sync.dma_start(cbf16_sb[:], cbf16_t[:])
            nc.sync.dma_start(cfp8_sb[:], cfp8_t[:])
            nc.sync.dma_start(cfp16_sb[:], cfp16_t[:])

            wq_sb = cf32_sb[:, 0:512]        # [:, (ec*2+dc)*128 + d]
            query_sb = cf32_sb[:, 512:576]   # [:, ec*32 + b]
            cbias_sb = cf32_sb[:, 576:578]
            ratio_sb = cf32_sb[:, 578:579]   # v1/v0 per pair-partition
            wref_sb = cbf16_sb[:, 0:512]     # bf16 W [(ec*2+dc)*128 + d]
            w8_sb = cfp8_sb[:, 0:512]        # fp8 W pairs [dc*256 + i*128 + m]
            v0_sb = cfp16_sb[:, 0:1]         # fp16 |v| even-rank column

            # ---- enc slab loading ----
            # fp8 b-rows first (slabs of 4 b), then bf16 b-rows
            def alloc_slab8(s):
                return encp.tile([128, 2 * SLAB_N], dt.float8e4, tag="enc8",
                                 name=f"enc8_s{s}")

            def emit_pieces8(t8, s, q0, q1, pieces=SLAB_B, dep=None):
                w = SLAB_N // pieces
                for q in range(q0, q1):
                    for ec in range(2):
                        ins = nc.sync.dma_start(
                            t8[:, ec * SLAB_N + q * w:ec * SLAB_N + (q + 1) * w],
                            enc8_t[ec * 128:(ec + 1) * 128,
                                   s * SLAB_N + q * w:s * SLAB_N + (q + 1) * w])
                        if dep is not None:
                            tile.add_dep_helper(ins.ins, dep.ins,
                                                reason="defer enc prefetch")

            def alloc_slabb(s):
                return [encp.tile([128, SLAB_N], dt.bfloat16, tag=f"encb{ec}",
                                  name=f"encb{ec}_s{s}")
                        for ec in range(2)]

            def emit_piecesb(tiles, s, q0, q1, pieces=SLAB_B, dep=None):
                w = SLAB_N // pieces
                for q in range(q0, q1):
                    for ec in range(2):
                        ins = nc.sync.dma_start(
                            tiles[ec][:, q * w:(q + 1) * w],
                            encb_t[ec * 128:(ec + 1) * 128,
                                   s * SLAB_N + q * w:s * SLAB_N + (q + 1) * w])
                        if dep is not None:
                            tile.add_dep_helper(ins.ins, dep.ins,
                                                reason="defer enc prefetch")

            slab0 = alloc_slab8(0)
            emit_pieces8(slab0, 0, 0, 1)

            # ---- q_rawT = (query @ Wq'.T).T per dc-chunk, + (bref' + bq') ----
            for dc in range(2):
                qps = psp.tile([128, BL], dt.float32, tag="st")
                for ec in range(2):
                    nc.tensor.matmul(
                        qps[:],
                        wq_sb[:, (ec * 2 + dc) * 128:(ec * 2 + dc + 1) * 128],
                        query_sb[:, ec * BL:(ec + 1) * BL],
                        start=(ec == 0), stop=(ec == 1),
                    )
                nc.vector.tensor_scalar_add(bias_sb[:, dc * BL:(dc + 1) * BL],
                                            qps[:], cbias_sb[:, dc:dc + 1])

            # ---- per-(b, kp) tail: combine + strips ----
            def emit_w(tts, b):
                # w' = t0 + ratio * t1  (TS 2x + TT 2x), fp16
                w1 = tp.tile([128, 1024], dt.float16, tag="w1", bufs=3)
                nc.vector.tensor_scalar_mul(w1[:], tts[1][:], ratio_sb[:, 0:1])
                w = tp.tile([128, 1024], dt.float16, tag="w", bufs=3)
                nc.vector.tensor_add(w[:], w1[:], tts[0][:])
                return w

            def emit_strips(st4, w, b, kp):
                for kb in range(2):
                    jj = kp * 2 + kb
                    nc.tensor.matmul(
                        st4[32 * jj:32 * jj + 1, :],
                        v0_sb,
                        w[:, kb * 512:(kb + 1) * 512],
                        start=True, stop=True,
                        skip_group_check=True,
                        tile_position=(0, 32 * jj),
                    )
                if kp == 1:
                    sp = tailp.tile([128, 512], dt.float32, tag="sp")
                    nc.vector.tensor_copy(sp[:], st4[:])
                    nc.gpsimd.dma_start(u_sb[4 * b:4 * b + 4, :],
                                        sp[0:128:32, :])

            t6 = constp.tile([128, 512], dt.float32)
            o6 = constp.tile([128, 512], dt.float32)

            def emit_final(half):
                rows = slice(64 * half, 64 * half + 64)
                nc.scalar.activation(t6[rows, :], u_sb[rows, :], AF.Tanh)
                nc.vector.tensor_scalar_mul(o6[rows, :], t6[rows, :], C_CLIP)
                nc.sync.dma_start(out_p[rows, :], o6[rows, :])

            pend = []
            prev_mm = None

            def tail_pump():
                while len(pend) > 2:
                    emit_strips(*pend.pop(0))

            # ---- unified main loop: fp8 / bf16 slabs interleaved ----
            NSLAB8 = N8 // SLAB_N
            NSLABB = NB // SLAB_N
            SLAB_SEQ = [("8", 0), ("b", 0), ("8", 1), ("b", 1), ("8", 2),
                        ("b", 2), ("b", 3), ("b", 4)]
            assert NSLAB8 == 3 and NSLABB == 5

            def alloc_and_prefetch(si, dep):
                typ, s = SLAB_SEQ[si]
                if typ == "8":
                    t = alloc_slab8(s)
                    emit_pieces8(t, s, 0, 2, 2, dep=dep)
                else:
                    t = alloc_slabb(s)
                    emit_piecesb(t, s, 0, 2, 2, dep=dep)
                return t

            cur_slab = slab0
            for si, (typ, s) in enumerate(SLAB_SEQ):
                nxt_slab = None
                for b_in in range(SLAB_B):
                    b = (SLAB_B * s + b_in) if typ == "8" else (NB8 + SLAB_B * s + b_in)
                    st4 = psp.tile([128, 512], dt.float32, tag="st")
                    if typ == "8":
                        enc_pair = cur_slab[:].rearrange("p (two n) -> p two n", two=2)
                    for kp in range(2):
                        if si == 0 and (b_in, kp) == (0, 1):
                            emit_pieces8(cur_slab, 0, 1, 2, dep=prev_mm)
                        if si == 0 and (b_in, kp) == (1, 0):
                            emit_pieces8(cur_slab, 0, 2, 4, dep=prev_mm)
                        pf_at = 2 if si == 0 else 1
                        if (b_in, kp) == (pf_at, 0) and si + 1 < len(SLAB_SEQ):
                            nxt_slab = alloc_and_prefetch(si + 1, prev_mm)
                        first_mm = None
                        tts = []
                        for dc in range(2):
                            psd = pmp.tile([128, 1024], dt.float32, tag="psd")
                            if typ == "8":
                                for kb in range(4):
                                    nseg = b_in * K + kp * 1024 + kb * 256
                                    ins = nc.tensor.matmul(
                                        psd[:, kb * 256:(kb + 1) * 256],
                                        w8_sb[:, dc * 256:(dc + 1) * 256].rearrange(
                                            "p (two m) -> p two m", two=2),
                                        enc_pair[:, :, nseg:nseg + 256],
                                        start=True, stop=True,
                                        perf_mode=mybir.MatmulPerfMode.DoubleRow,
                                        skip_group_check=True,
                                    )
                                    if first_mm is None:
                                        first_mm = ins
                            else:
                                for ec in range(2):
                                    for kb in range(2):
                                        nseg = b_in * K + kp * 1024 + kb * 512
                                        ins = nc.tensor.matmul(
                                            psd[:, kb * 512:(kb + 1) * 512],
                                            wref_sb[:, (ec * 2 + dc) * 128:(ec * 2 + dc + 1) * 128],
                                            cur_slab[ec][:, nseg:nseg + 512],
                                            start=(ec == 0), stop=(ec == 1),
                                            skip_group_check=True,
                                        )
                                        if first_mm is None:
                                            first_mm = ins
                            ttile = tp.tile([128, 1024], dt.float16, tag="tt")
                            nc.scalar.activation(
                                ttile[:], psd[:], AF.Tanh,
                                bias=bias_sb[:, dc * BL + b:dc * BL + b + 1],
                                scale=(DESCALE if typ == "8" else 1.0))
                            tts.append(ttile)
                        w = emit_w(tts, b)
                        pend.append((st4, w, b, kp))
                        tail_pump()
                        prev_mm = first_mm
                        if (si, b_in, kp) == (5, 1, 0):
                            emit_final(0)
                cur_slab = nxt_slab
            for args in pend:
                emit_strips(*args)
            emit_final(1)

    nc.compile()
    return nc


def _prep_inputs(encoder_output, query, Wq, bq, Wref, bref, v):
    bf16 = ml_dtypes.bfloat16
    fp16 = np.float16
    e4 = ml_dtypes.float8_e4m3fn if hasattr(ml_dtypes, "float8_e4m3fn") else ml_dtypes.float8_e4m3

    v = np.asarray(v, np.float32)
    sgn = np.where(v >= 0, 1.0, -1.0).astype(np.float32)
    va = np.abs(v)
    order = np.argsort(va, kind="stable")
    c0_idx, c1_idx = order[0::2], order[1::2]
    v0, v1 = va[c0_idx], va[c1_idx]
    ratio = (v1 / v0).astype(np.float32)
    perm = np.concatenate([c0_idx, c1_idx])          # new d order (dc-major)

    Wp = (np.asarray(Wref, np.float32) * sgn[:, None])[perm]     # (256, 256)
    Wqp = (np.asarray(Wq, np.float32) * sgn[:, None])[perm]
    cbias = (np.asarray(bref, np.float32) + np.asarray(bq, np.float32)) * sgn
    cbias = cbias[perm]

    def chunk4(w):                                   # (E, 256d) -> (512, 128)
        return np.ascontiguousarray(
            w.reshape(2, 128, 2, 128).transpose(0, 2, 1, 3).reshape(512, 128))

    def pack(w4):                                    # (4*128, X) -> (128, 4*X)
        x = w4.shape[1]
        return w4.reshape(4, 128, x).transpose(1, 0, 2).reshape(128, 4 * x)

    # bf16 W pack: WT (E, D') where D' columns are [c0 | c1]
    WT = np.ascontiguousarray(Wp.T)                  # (E, 256) cols dc-major
    wref_p = pack(chunk4(WT)).astype(bf16)           # (128, 512)
    wq_p = pack(chunk4(np.ascontiguousarray(Wqp.T))) # (128, 512) f32

    # fp8 W pairs: per dc, lhsT[e, i, m] = Wp[dc*128+m, i*128+e] * S_W
    w8 = np.empty((128, 512), np.float32)
    for dc in range(2):
        chunk = Wp[dc * 128:(dc + 1) * 128] * S_W    # (128 d, 256 e)
        for i in range(2):
            # (e, m) block
            w8[:, dc * 256 + i * 128:dc * 256 + (i + 1) * 128] = \
                chunk[:, i * 128:(i + 1) * 128].T
    w8 = np.clip(w8, -240.0, 240.0).astype(e4)

    cbias_p = cbias.reshape(2, 128).T                # (128, 2)
    ratio_p = ratio.reshape(128, 1)
    v0_p = v0.reshape(128, 1).astype(fp16)
    queryT = np.ascontiguousarray(np.asarray(query, np.float32).T)  # (E, B)

    enc = np.asarray(encoder_output, np.float32)     # (K, B, E)
    encT = enc.transpose(2, 1, 0)                    # (E, B, K) view

    in_maps = []
    for c in range(NCORES):
        bs = slice(c * BL, (c + 1) * BL)
        enc_c = encT[:, bs, :]                       # (E, 32, K)
        enc8 = np.ascontiguousarray(enc_c[:, 0:NB8, :]).reshape(E, N8)
        enc8 = np.clip(enc8 * S_ENC, -240.0, 240.0).astype(e4)
        encb = np.ascontiguousarray(enc_c[:, NB8:, :]).reshape(E, NB).astype(bf16)

        q_c = queryT[:, bs]                          # (256, 32)
        q_p = q_c.reshape(2, 128, BL).transpose(1, 0, 2).reshape(128, 2 * BL)
        cf32 = np.ascontiguousarray(np.concatenate(
            [wq_p, q_p, cbias_p, ratio_p,
             np.zeros((128, 2), np.float32)], axis=1), dtype=np.float32)
        in_maps.append({
            "enc8": enc8,
            "encb": encb,
            "cf32": cf32,
            "cbf16": wref_p,
            "cfp8": w8,
            "cfp16": v0_p,
        })
    return in_maps


def kernel(**inputs):
    global _compiled, last_exec_time_ns, last_results
    from concourse import bass_utils

    if _compiled is None:
        _compiled = _build()
    nc = _compiled

    in_maps = _prep_inputs(**inputs)
    res = bass_utils.run_bass_kernel_spmd(nc, in_maps, core_ids=list(range(NCORES)))
    last_exec_time_ns = res.exec_time_ns
    last_results = res
    out = np.concatenate(
        [r["out"].reshape(BL, K) for r in res.results], axis=0)
    return out


# revision 5
# speedup vs baseline: 1.0051x; 1.0025x over previous
"""Trainium2 Bass kernel v2 for the attention-scoring module.

    q = query @ Wq.T + bq                               # (B, D)
    ref[b,d,k] = sum_e enc[k,b,e] * Wref[d,e] + bref[d]
    u[b,k] = sum_d v[d] * tanh(ref[b,d,k] + q[b,d])
    out = 10 * tanh(u)                                  # (B, K)

Data-parallel over batch: core c owns b in [32c, 32c+32).

v2 changes vs baseline:
  - sign(v) folded into Wref/Wq/bref/bq rows (tanh odd), then d's sorted
    by |v| and paired (rank 2i, 2i+1) -> chunks c0/c1 with ratio=v1/v0
    in [1, ~1.9].  The d-combine is then w' = t0 + ratio*t1 (one
    tensor_scalar + one tensor_tensor, both 2x) and the strip matmul's
    stationary column is v0 (not ones) - halves DVE combine cost.
  - 16 of 32 b-rows use fp8(e4m3) DoubleRow matmuls: contraction 256 in
    one instruction at 2x rate; enc scaled x16, Wref x512, descale via
    the activation's scale input.  Error budget ~1.6e-2 < 2e-2.
  - t/w tiles fp16 (same speed, more precision margin than bf16).
"""

import os
import sys

import numpy as np

os.environ.setdefault("JAX_COMPILATION_CACHE_DIR", "/tmp/jaxcache")

for _p in ("/opt/trn_rl_repo", "/opt/pypackages"):
    if _p not in sys.path:
        sys.path.append(_p)

import ml_dtypes

E = 256
D = 256
K = 2048
B = 256
NCORES = 8
BL = B // NCORES          # 32 batch rows per core
NB8 = 12                  # b-rows (per core) on the fp8 path
NBB = BL - NB8            # b-rows on the bf16 path
N8 = NB8 * K
NB = NBB * K
SLAB_B = 4                # b-rows per enc DMA slab
SLAB_N = SLAB_B * K       # 8192
C_CLIP = 10.0
S_ENC = 16.0
S_W = 512.0
DESCALE = 1.0 / (S_ENC * S_W)

_compiled = None
last_exec_time_ns = None
last_results = None


def _build():
    from concourse import bacc, bass, tile
    from concourse.alu_op_type import AluOpType as ALU

    mybir = bass.mybir
    dt = mybir.dt
    AF = mybir.ActivationFunctionType

    nc = bacc.Bacc("TRN2", target_bir_lowering=False, debug=False,
                   num_devices=NCORES)

    # fp8 enc for b 0..15: rows 0:128 = stream A (e 0:128), 128:256 = B
    enc8_t = nc.declare_dram_parameter("enc8", [E, N8], dt.float8e4, isOutput=False)
    # bf16 enc for b 16..31, two row-halves like the baseline
    encb_t = nc.declare_dram_parameter("encb", [E, NB], dt.bfloat16, isOutput=False)
    cf32_t = nc.declare_dram_parameter("cf32", [128, 581], dt.float32, isOutput=False)
    cbf16_t = nc.declare_dram_parameter("cbf16", [128, 512], dt.bfloat16, isOutput=False)
    cfp8_t = nc.declare_dram_parameter("cfp8", [128, 512], dt.float8e4, isOutput=False)
    cfp16_t = nc.declare_dram_parameter("cfp16", [128, 1], dt.float16, isOutput=False)
    out_p = nc.declare_dram_parameter("out", [128, 512], dt.float32, isOutput=True)

    with tile.TileContext(nc) as tc:
        with (
            tc.tile_pool(name="const", bufs=1) as constp,
            tc.tile_pool(name="enc", bufs=3) as encp,
            tc.tile_pool(name="tt", bufs=10) as tp,
            tc.tile_pool(name="tail", bufs=2) as tailp,
            tc.tile_pool(name="psum_m", bufs=3, space="PSUM") as pmp,
            tc.tile_pool(name="psum_s", bufs=2, space="PSUM") as psp,
        ):
            # ---- constants: packed DMAs ----
            cf32_sb = constp.tile([128, 581], dt.float32)
            cbf16_sb = constp.tile([128, 512], dt.bfloat16)
            cfp8_sb = constp.tile([128, 512], dt.float8e4)
            cfp16_sb = constp.tile([128, 1], dt.float16)
            bias_sb = constp.tile([128, 2 * BL], dt.float32)   # [:, dc*32 + b]
            u_sb = constp.tile([128, 512], dt.float32)         # [b*4+jj, kk]
            nc.sync.dma_start(cf32_sb[:], cf32_t[:])
            nc.sync.dma_start(cbf16_sb[:], cbf16_t[:])
            nc.sync.dma_start(cfp8_sb[:], cfp8_t[:])
            nc.sync.dma_start(cfp16_sb[:], cfp16_t[:])

            wq_sb = cf32_sb[:, 0:512]        # [:, (ec*2+dc)*128 + d]
            query_sb = cf32_sb[:, 512:576]   # [:, ec*32 + b]
            cbias_sb = cf32_sb[:, 576:578]
            ratio_sb = cf32_sb[:, 578:579]   # v1/v0 per pair-partition
            wref_sb = cbf16_sb[:, 0:512]     # bf16 W [(ec*2+dc)*128 + d]
            w8_sb = cfp8_sb[:, 0:512]        # fp8 W pairs [dc*256 + i*128 + m]
            v0_sb = cfp16_sb[:, 0:1]         # fp16 |v| even-rank column

            # ---- enc slab loading ----
            # fp8 b-rows first (slabs of 4 b), then bf16 b-rows
            def alloc_slab8(s):
                return encp.tile([128, 2 * SLAB_N], dt.float8e4, tag="enc8",
                                 name=f"enc8_s{s}")

            def emit_pieces8(t8, s, q0, q1, pieces=SLAB_B, dep=None):
                w = SLAB_N // pieces
                for q in range(q0, q1):
                    for ec in range(2):
                        ins = nc.sync.dma_start(
                            t8[:, ec * SLAB_N + q * w:ec * SLAB_N + (q + 1) * w],
                            enc8_t[ec * 128:(ec + 1) * 128,
                                   s * SLAB_N + q * w:s * SLAB_N + (q + 1) * w])
                        if dep is not None:
                            tile.add_dep_helper(ins.ins, dep.ins,
                                                reason="defer enc prefetch")

            def alloc_slabb(s):
                return [encp.tile([128, SLAB_N], dt.bfloat16, tag=f"encb{ec}",
                                  name=f"encb{ec}_s{s}")
                        for ec in range(2)]

            def emit_piecesb(tiles, s, q0, q1, pieces=SLAB_B, dep=None):
                w = SLAB_N // pieces
                for q in range(q0, q1):
                    for ec in range(2):
                        ins = nc.sync.dma_start(
                            tiles[ec][:, q * w:(q + 1) * w],
                            encb_t[ec * 128:(ec + 1) * 128,
                                   s * SLAB_N + q * w:s * SLAB_N + (q + 1) * w])
                        if dep is not None:
                            tile.add_dep_helper(ins.ins, dep.ins,
                                                reason="defer enc prefetch")

            slab0 = alloc_slab8(0)
            emit_pieces8(slab0, 0, 0, 1)

            # ---- q_rawT = (query @ Wq'.T).T per dc-chunk, + (bref' + bq') ----
            for dc in range(2):
                qps = psp.tile([128, BL], dt.float32, tag="st")
                for ec in range(2):
                    nc.tensor.matmul(
                        qps[:],
                        wq_sb[:, (ec * 2 + dc) * 128:(ec * 2 + dc + 1) * 128],
                        query_sb[:, ec * BL:(ec + 1) * BL],
                        start=(ec == 0), stop=(ec == 1),
                    )
                nc.vector.tensor_scalar_add(bias_sb[:, dc * BL:(dc + 1) * BL],
                                            qps[:], cbias_sb[:, dc:dc + 1])

            # ---- per-b tail: combine over both kp halves at once ----
            def emit_w(tts, b):
                # w' = t0 + ratio * t1  (TS 2x + TT 2x), fp16, 2048 wide
                w1 = tp.tile([128, 2048], dt.float16, tag="w1", bufs=2)
                nc.vector.tensor_scalar_mul(w1[:], tts[1][:], ratio_sb[:, 0:1])
                w = tp.tile([128, 2048], dt.float16, tag="w", bufs=2)
                nc.vector.tensor_add(w[:], w1[:], tts[0][:])
                return w

            def emit_strips(st4, w, b):
                for jj in range(4):
                    nc.tensor.matmul(
                        st4[32 * jj:32 * jj + 1, :],
                        v0_sb,
                        w[:, jj * 512:(jj + 1) * 512],
                        start=True, stop=True,
                        skip_group_check=True,
                        tile_position=(0, 32 * jj),
                    )
                sp = tailp.tile([128, 512], dt.float32, tag="sp")
                nc.vector.tensor_copy(sp[:], st4[:])
                nc.gpsimd.dma_start(u_sb[4 * b:4 * b + 4, :],
                                    sp[0:128:32, :])

            t6 = constp.tile([128, 512], dt.float32)
            o6 = constp.tile([128, 512], dt.float32)

            def emit_final(half):
                rows = slice(64 * half, 64 * half + 64)
                nc.scalar.activation(t6[rows, :], u_sb[rows, :], AF.Tanh)
                nc.vector.tensor_scalar_mul(o6[rows, :], t6[rows, :], C_CLIP)
                nc.sync.dma_start(out_p[rows, :], o6[rows, :])

            pend = []
            prev_mm = None

            def tail_pump():
                while len(pend) > 1:
                    emit_strips(*pend.pop(0))

            # ---- unified main loop: fp8 / bf16 slabs interleaved ----
            NSLAB8 = N8 // SLAB_N
            NSLABB = NB // SLAB_N
            SLAB_SEQ = [("8", 0), ("b", 0), ("8", 1), ("b", 1), ("8", 2),
                        ("b", 2), ("b", 3), ("b", 4)]
            assert NSLAB8 == 3 and NSLABB == 5

            def alloc_and_prefetch(si, dep):
                typ, s = SLAB_SEQ[si]
                if typ == "8":
                    t = alloc_slab8(s)
                    emit_pieces8(t, s, 0, 4, 4, dep=dep)
                else:
                    t = alloc_slabb(s)
                    emit_piecesb(t, s, 0, 4, 4, dep=dep)
                return t

            cur_slab = slab0
            for si, (typ, s) in enumerate(SLAB_SEQ):
                nxt_slab = None
                for b_in in range(SLAB_B):
                    b = (SLAB_B * s + b_in) if typ == "8" else (NB8 + SLAB_B * s + b_in)
                    st4 = psp.tile([128, 512], dt.float32, tag="st")
                    btts = [tp.tile([128, 2048], dt.float16, tag="tt", bufs=5,
                                    name=f"tt_{b}_{dcx}")
                            for dcx in range(2)]
                    if typ == "8":
                        enc_pair = cur_slab[:].rearrange("p (two n) -> p two n", two=2)
                    for kp in range(2):
                        if si == 0 and (b_in, kp) == (0, 1):
                            emit_pieces8(cur_slab, 0, 1, 2, dep=prev_mm)
                        if si == 0 and (b_in, kp) == (1, 0):
                            emit_pieces8(cur_slab, 0, 2, 4, dep=prev_mm)
                        pf_now = ((b_in, kp) == (2, 0)) if si == 0 else \
                                 ((b_in, kp) == (0, 1))
                        if pf_now and si + 1 < len(SLAB_SEQ):
                            nxt_slab = alloc_and_prefetch(si + 1, prev_mm)
                        first_mm = None
                        for dc in range(2):
                            psd = pmp.tile([128, 1024], dt.float32, tag="psd")
                            if typ == "8":
                                for kb in range(4):
                                    nseg = b_in * K + kp * 1024 + kb * 256
                                    ins = nc.tensor.matmul(
                                        psd[:, kb * 256:(kb + 1) * 256],
                                        w8_sb[:, dc * 256:(dc + 1) * 256].rearrange(
                                            "p (two m) -> p two m", two=2),
                                        enc_pair[:, :, nseg:nseg + 256],
                                        start=True, stop=True,
                                        perf_mode=mybir.MatmulPerfMode.DoubleRow,
                                        skip_group_check=True,
                                    )
                                    if first_mm is None:
                                        first_mm = ins
                            else:
                                for ec in range(2):
                                    for kb in range(2):
                                        nseg = b_in * K + kp * 1024 + kb * 512
                                        ins = nc.tensor.matmul(
                                            psd[:, kb * 512:(kb + 1) * 512],
                                            wref_sb[:, (ec * 2 + dc) * 128:(ec * 2 + dc + 1) * 128],
                                            cur_slab[ec][:, nseg:nseg + 512],
                                            start=(ec == 0), stop=(ec == 1),
                                            skip_group_check=True,
                                        )
                                        if first_mm is None:
                                            first_mm = ins
                            nc.scalar.activation(
                                btts[dc][:, kp * 1024:(kp + 1) * 1024],
                                psd[:], AF.Tanh,
                                bias=bias_sb[:, dc * BL + b:dc * BL + b + 1],
                                scale=(DESCALE if typ == "8" else 1.0))
                        if kp == 1:
                            w = emit_w(btts, b)
                            pend.append((st4, w, b))
                            tail_pump()
                        prev_mm = first_mm
                        if (si, b_in, kp) == (5, 1, 0):
                            emit_final(0)
                cur_slab = nxt_slab
            for args in pend:
                emit_strips(*args)
            emit_final(1)

    nc.compile()
    return nc


def _prep_inputs(encoder_output, query, Wq, bq, Wref, bref, v):
    bf16 = ml_dtypes.bfloat16
    fp16 = np.float16
    e4 = ml_dtypes.float8_e4m3fn if hasattr(ml_dtypes, "float8_e4m3fn") else ml_dtypes.float8_e4m3

    v = np.asarray(v, np.float32)
    sgn = np.where(v >= 0, 1.0, -1.0).astype(np.float32)
    va = np.abs(v)
    order = np.argsort(va, kind="stable")
    c0_idx, c1_idx = order[0::2], order[1::2]
    v0, v1 = va[c0_idx], va[c1_idx]
    ratio = (v1 / v0).astype(np.float32)
    perm = np.concatenate([c0_idx, c1_idx])          # new d order (dc-major)

    Wp = (np.asarray(Wref, np.float32) * sgn[:, None])[perm]     # (256, 256)
    Wqp = (np.asarray(Wq, np.float32) * sgn[:, None])[perm]
    cbias = (np.asarray(bref, np.float32) + np.asarray(bq, np.float32)) * sgn
    cbias = cbias[perm]

    def chunk4(w):                                   # (E, 256d) -> (512, 128)
        return np.ascontiguousarray(
            w.reshape(2, 128, 2, 128).transpose(0, 2, 1, 3).reshape(512, 128))

    def pack(w4):                                    # (4*128, X) -> (128, 4*X)
        x = w4.shape[1]
        return w4.reshape(4, 128, x).transpose(1, 0, 2).reshape(128, 4 * x)

    # bf16 W pack: WT (E, D') where D' columns are [c0 | c1]
    WT = np.ascontiguousarray(Wp.T)                  # (E, 256) cols dc-major
    wref_p = pack(chunk4(WT)).astype(bf16)           # (128, 512)
    wq_p = pack(chunk4(np.ascontiguousarray(Wqp.T))) # (128, 512) f32

    # fp8 W pairs: per dc, lhsT[e, i, m] = Wp[dc*128+m, i*128+e] * S_W
    w8 = np.empty((128, 512), np.float32)
    for dc in range(2):
        chunk = Wp[dc * 128:(dc + 1) * 128] * S_W    # (128 d, 256 e)
        for i in range(2):
            # (e, m) block
            w8[:, dc * 256 + i * 128:dc * 256 + (i + 1) * 128] = \
                chunk[:, i * 128:(i + 1) * 128].T
    w8 = np.clip(w8, -240.0, 240.0).astype(e4)

    cbias_p = cbias.reshape(2, 128).T                # (128, 2)
    ratio_p = ratio.reshape(128, 1)
    v0_p = v0.reshape(128, 1).astype(fp16)
    queryT = np.ascontiguousarray(np.asarray(query, np.float32).T)  # (E, B)

    enc = np.asarray(encoder_output, np.float32)     # (K, B, E)
    encT = enc.transpose(2, 1, 0)                    # (E, B, K) view

    in_maps = []
    for c in range(NCORES):
        bs = slice(c * BL, (c + 1) * BL)
        enc_c = encT[:, bs, :]                       # (E, 32, K)
        enc8 = np.ascontiguousarray(enc_c[:, 0:NB8, :]).reshape(E, N8)
        enc8 = np.clip(enc8 * S_ENC, -240.0, 240.0).astype(e4)
        encb = np.ascontiguousarray(enc_c[:, NB8:, :]).reshape(E, NB).astype(bf16)

        q_c = queryT[:, bs]                          # (256, 32)
        q_p = q_c.reshape(2, 128, BL).transpose(1, 0, 2).reshape(128, 2 * BL)
        cf32 = np.ascontiguousarray(np.concatenate(
            [wq_p, q_p, cbias_p, ratio_p,
             np.zeros((128, 2), np.float32)], axis=1), dtype=np.float32)
        in_maps.append({
            "enc8": enc8,
            "encb": encb,
            "cf32": cf32,
            "cbf16": wref_p,
            "cfp8": w8,
            "cfp16": v0_p,
        })
    return in_maps


def kernel(**inputs):
    global _compiled, last_exec_time_ns, last_results
    from concourse import bass_utils

    if _compiled is None:
        _compiled = _build()
    nc = _compiled

    in_maps = _prep_inputs(**inputs)
    res = bass_utils.run_bass_kernel_spmd(nc, in_maps, core_ids=list(range(NCORES)))
    last_exec_time_ns = res.exec_time_ns
    last_results = res
    out = np.concatenate(
        [r["out"].reshape(BL, K) for r in res.results], axis=0)
    return out
